# revision 19
# baseline (speedup 1.0000x reference)
"""Trainium2 Bass kernel for nn_CapsuleEncoder (conv stem -> primary caps conv
-> squash -> per-batch routing einsum -> dynamic routing).

Sharding over 8 NeuronCores:
  - conv1 (3->128, 9x9 s1) replicated on every core (tiny).
  - conv2 (128->1024, 9x9 s2) sharded over output channels: 128 couts/core.
    This makes each core own exactly the slice of the routing-einsum
    contraction dim c = (cout, oh) it needs -> tensor-parallel einsum.
  - Partial t = einsum(u, caps_w) AllReduce'd (262 KB) across the 8 cores.
  - Dynamic routing (tiny, 16^4 tensors) replicated on every core.
All matmuls run as float32r (TF32-like, full PE rate at free-dim >= 256).
"""

import sys

sys.path.insert(0, "/opt/trn_rl_repo")

import numpy as np

import concourse.bass as bass
import concourse.bacc as bacc
import concourse.mybir as mybir
from concourse import tile
from concourse import bass_utils

F32 = mybir.dt.float32
F32R = mybir.dt.float32r
BF16 = mybir.dt.bfloat16
AX = mybir.AxisListType
ALU = mybir.AluOpType
ACTF = mybir.ActivationFunctionType

NCORES = 8
B = 16          # batch (== num capsules K by the module's tying)
CI1, CO1 = 3, 128
H0 = 47
KH = KW = 9
H1 = 39         # conv1 output spatial
C2 = 1024       # total primary-caps channels
COL = C2 // NCORES  # 128 couts per core
OH = OW = 16    # conv2 output spatial
KN = 16         # routing out-capsules (einsum 'n')
D = 16          # capsule dim
GB = 2          # batches per conv2/einsum group
NG = B // GB    # 8 groups

# conv1 im2col scheme: partitions = (ci, kh, kwg) with kwg in 0..2 covering
# kw = 3*kwg + j for pass j in 0..2;  81 partitions, 3 accumulating passes.
NKWG = 3
P81 = CI1 * KH * NKWG  # 81
JJ = 42                # im2col row length (padded so matmul N stays even)
OWP = 40               # conv1 padded ow per matmul (fp32r needs even innermost)
XPAD = 64              # tail padding on flat x (im2col DMA over-reads)


def _build_program():
    nc = bacc.Bacc("TRN2", target_bir_lowering=False, debug=False,
                   enable_asserts=False, num_devices=NCORES)

    x_d = nc.dram_tensor("x", [B * CI1 * H0 * H0 + XPAD], F32,
                         kind="ExternalInput")
    w1_d = nc.dram_tensor("w1t", [NKWG, P81, CO1], F32, kind="ExternalInput")
    b1_d = nc.dram_tensor("b1", [CO1], F32, kind="ExternalInput")
    w2_d = nc.dram_tensor("w2t", [128, KH * KW * COL], F32, kind="ExternalInput")
    b2_d = nc.dram_tensor("b2", [COL], F32, kind="ExternalInput")
    cw_d = nc.dram_tensor("capst", [B, OH, COL, KN * D], BF16,
                          kind="ExternalInput")
    out_d = nc.dram_tensor("out", [B, KN, D], F32, kind="ExternalOutput")

    with tile.TileContext(nc) as tc:
        _build_kernel(nc, tc, x_d, w1_d, b1_d, w2_d, b2_d, cw_d, out_d)
    nc.finalize()
    return nc


def _build_kernel(nc, tc, x_d, w1_d, b1_d, w2_d, b2_d, cw_d, out_d):
    fr = lambda ap: ap.bitcast(F32R)

    with (
        tc.tile_pool(name="const", bufs=1) as const_pool,
        tc.tile_pool(name="usq", bufs=3) as usq_pool,
        tc.tile_pool(name="caps", bufs=20) as caps_pool,
        tc.tile_pool(name="sq", bufs=2) as sq_pool,
        tc.tile_pool(name="tsb", bufs=1) as tsb_pool,
        tc.tile_pool(name="ps1", bufs=2, space="PSUM") as ps1_pool,
        tc.tile_pool(name="ps2", bufs=2, space="PSUM") as ps2_pool,
        tc.tile_pool(name="pst", bufs=2, space="PSUM") as pst_pool,
        tc.tile_pool(name="dram", bufs=1, space="DRAM") as dram_pool,
    ):
        # ---------------- constants ----------------
        w1_sb = const_pool.tile([P81, NKWG * CO1], F32R)   # [81, 3*128]
        # DRAM [j, p, co] -> SBUF [p, (j, co)]
        nc.sync.dma_start(
            w1_sb[:].rearrange("p (j co) -> p j co", j=NKWG, co=CO1),
            w1_d.ap().transpose([1, 0, 2]).bitcast(F32R))
        b1_sb = const_pool.tile([CO1, 1], F32)
        nc.sync.dma_start(b1_sb[:], b1_d.ap().unsqueeze(1))
        b2_sb = const_pool.tile([COL, 1], F32)
        nc.sync.dma_start(b2_sb[:], b2_d.ap().unsqueeze(1))

        t_sb = tsb_pool.tile([OW, B * KN * D], F32)  # [w=16, (b, k, d)]

        with tc.tile_pool(name="conv", bufs=1) as conv_pool:
            w2_sb = conv_pool.tile([128, KH * KW * COL], F32R)   # 5.3 MB
            nc.sync.dma_start(w2_sb[:], w2_d.ap().bitcast(F32R))

            # ---------------- conv1 (replicated, all 16 batches) ----------
            # y_sb [ci2=128, (b, ih, iw)] : conv2's contraction layout
            y_sb = conv_pool.tile([CO1, B * H1 * H1], F32R)      # 12.2 MB
            y4 = y_sb[:].rearrange("p (b h w) -> p b h w", b=B, h=H1, w=H1)

            with tc.tile_pool(name="z81", bufs=2) as z_pool:
                for b0 in range(B):
                    z = z_pool.tile([P81, H1 * JJ], F32R, tag="z81")
                    # im2col DMAs: one per (ci, kh) -> 3 kwg partitions each
                    for ci in range(CI1):
                        for kh in range(KH):
                            p0 = ci * (KH * NKWG) + kh * NKWG
                            src = bass.AP(
                                x_d,
                                b0 * (CI1 * H0 * H0) + ci * (H0 * H0) + kh * H0,
                                [[3, NKWG], [H0, H1], [1, JJ]])
                            nc.sync.dma_start(z[p0:p0 + NKWG, :], src.bitcast(F32R))
                    zv = z[:].rearrange("p (oh jj) -> p oh jj", oh=H1, jj=JJ)
                    oh0 = 0
                    for ohc in (12, 12, 12, 3):
                        ps = ps1_pool.tile([CO1, 12 * OWP], F32, tag="ps1")
                        psv = ps[:].rearrange("p (o w) -> p o w", w=OWP)
                        for j in range(NKWG):
                            rhs = zv[:, oh0:oh0 + ohc, j:j + OWP]
                            nc.tensor.matmul(
                                psv[:, :ohc, :], w1_sb[:, j * CO1:(j + 1) * CO1],
                                rhs,
                                start=(j == 0), stop=(j == NKWG - 1))
                        off = b0 * (H1 * H1) + oh0 * H1
                        nc.scalar.activation(
                            y_sb[:, off:off + ohc * H1]
                            .rearrange("p (o w) -> p o w", w=H1),
                            psv[:, :ohc, :H1], ACTF.Relu,
                            bias=b1_sb[:])
                        oh0 += ohc

            # ---------------- conv2 + squash + einsum pipeline ------------
            usq_tiles = [None] * NG

            def conv2_group(g):
                b0 = g * GB
                ps2 = ps2_pool.tile([COL, GB * OH * OW], F32, tag="ps2")
                first = True
                for kh in range(KH):
                    for kw in range(KW):
                        rhs = y4[:, b0:b0 + GB,
                                 kh:kh + 2 * OH - 1:2, kw:kw + 2 * OW - 1:2]
                        wofs = (kh * KW + kw) * COL
                        nc.tensor.matmul(
                            ps2[:], w2_sb[:, wofs:wofs + COL], rhs,
                            start=first,
                            stop=(kh == KH - 1 and kw == KW - 1))
                        first = False
                # bias-add (psum -> sbuf), then squash over ow
                uw = usq_pool.tile([COL, GB * OH * OW], F32, tag="uw", bufs=2)
                nc.vector.tensor_scalar_add(uw[:], ps2[:], b2_sb[:])
                uw3 = uw[:].rearrange("p (r w) -> p r w", w=OW)  # r=(b,oh)=32
                sqt = sq_pool.tile([COL, GB * OH * OW], F32, tag="sqt")
                nc.vector.tensor_mul(sqt[:], uw[:], uw[:])
                sq = sq_pool.tile([COL, GB * OH], F32, tag="sq")
                nc.vector.tensor_reduce(
                    sq[:].unsqueeze(2),
                    sqt[:].rearrange("p (r w) -> p r w", w=OW), AX.X, ALU.add)
                rt = sq_pool.tile([COL, GB * OH], F32, tag="rtt")
                nc.scalar.activation(rt[:], sq[:], ACTF.Sqrt)
                dn = sq_pool.tile([COL, GB * OH], F32, tag="dn")
                nc.vector.tensor_scalar_add(dn[:], sq[:], 1.0)
                rc = sq_pool.tile([COL, GB * OH], F32, tag="rc")
                nc.vector.reciprocal(rc[:], dn[:])
                sc = sq_pool.tile([COL, GB * OH], F32, tag="sc")
                nc.vector.tensor_mul(sc[:], rt[:], rc[:])
                usq = usq_pool.tile([COL, GB * OH * OW], BF16, tag="usq", bufs=3)
                nc.vector.tensor_mul(
                    usq[:].rearrange("p (r w) -> p r w", w=OW), uw3,
                    sc[:].unsqueeze(2).broadcast_to([COL, GB * OH, OW]))
                usq_tiles[g] = usq

            def einsum_group(g):
                b0 = g * GB
                usq = usq_tiles[g]
                uv = usq[:].rearrange("p (bb oh w) -> p bb oh w",
                                      bb=GB, oh=OH, w=OW)
                for bb in range(GB):
                    b = b0 + bb
                    pt = pst_pool.tile([OW, KN * D], F32, tag="pst")
                    for oh in range(OH):
                        ct = caps_pool.tile([COL, KN * D], BF16, tag="caps",
                                            bufs=20)
                        nc.sync.dma_start(ct[:], cw_d.ap()[b, oh])
                        nc.tensor.matmul(
                            pt[:], uv[:, bb, oh, :], ct[:],
                            start=(oh == 0), stop=(oh == OH - 1))
                    nc.vector.tensor_copy(
                        t_sb[:, b * (KN * D):(b + 1) * (KN * D)], pt[:])

            for g in range(NG):
                conv2_group(g)
                if g >= 1:
                    einsum_group(g - 1)
            einsum_group(NG - 1)

        # ---------------- AllReduce partial t ----------------
        t_in = dram_pool.tile([B, OW * KN * D], F32)
        t_out = dram_pool.tile([B, OW * KN * D], F32, addr_space="Shared")
        # t_sb [w, (b,k,d)] -> DRAM [b, (w,k,d)]
        nc.sync.dma_start(
            t_in[:].rearrange("b (w kd) -> w b kd", w=OW, kd=KN * D),
            t_sb[:].rearrange("w (b kd) -> w b kd", b=B, kd=KN * D))
        nc.gpsimd.collective_compute(
            "AllReduce", ALU.add,
            replica_groups=[list(range(NCORES))],
            ins=[t_in[:]], outs=[t_out[:]])

        # ---------------- dynamic routing (replicated) ----------------
        with tc.tile_pool(name="rt", bufs=1) as rt_pool:
            # T_full[b][n][k][d] with n = spatial w. T2 [b(part), (k,n,d)],
            # free strides k:256, n:16, d:1.
            T2 = rt_pool.tile([B, KN * OW * D], F32)
            Traw = rt_pool.tile([B, OW * KN * D], F32)
            nc.sync.dma_start(Traw[:], t_out[:])
            nc.vector.tensor_copy(
                T2[:].rearrange("b (k n d) -> b n k d", k=KN, n=OW, d=D),
                Traw[:].rearrange("b (n k d) -> b n k d", n=OW, k=KN, d=D))
            T2knd = T2[:].rearrange("b (k n d) -> b k n d", k=KN, n=OW, d=D)
            T2kdn = T2knd.transpose([0, 1, 3, 2])

            L = rt_pool.tile([KN, OW * D], F32)    # logits(k,n,d); (n,d)=(16,1)
            P = rt_pool.tile([KN, OW * D], F32)    # probs, same layout
            Pf = rt_pool.tile([1, KN * OW * D], F32)
            Pr = rt_pool.tile([B, KN * OW * D], F32)  # probs bcast over b
            tmp = rt_pool.tile([B, KN * OW * D], F32)
            vr = rt_pool.tile([B, KN * D], F32)    # raw out, (k,d) = (16,1)
            v2 = rt_pool.tile([B, KN * D], F32)
            s2 = rt_pool.tile([B, KN * OW], F32)   # logit delta, (k,n)=(16,1)
            sqv = rt_pool.tile([B, KN], F32)
            rtv = rt_pool.tile([B, KN], F32)
            dnv = rt_pool.tile([B, KN], F32)
            rcv = rt_pool.tile([B, KN], F32)
            scv = rt_pool.tile([B, KN], F32)
            m1 = rt_pool.tile([KN, D], F32)
            e1 = rt_pool.tile([KN, OW * D], F32)
            z1 = rt_pool.tile([KN, D], F32)
            zr = rt_pool.tile([KN, D], F32)

            vr_kd = vr[:].rearrange("b (k d) -> b k d", k=KN, d=D)
            v2_kd = v2[:].rearrange("b (k d) -> b k d", k=KN, d=D)
            tmp_kdn = tmp[:].rearrange("b (k d n) -> b k d n", k=KN, d=D, n=OW)
            tmp_knd = tmp[:].rearrange("b (k n d) -> b k n d", k=KN, n=OW, d=D)
            L_nd = L[:].rearrange("k (n d) -> k n d", n=OW, d=D)
            L_dn = L_nd.transpose([0, 2, 1])
            P_nd = P[:].rearrange("k (n d) -> k n d", n=OW, d=D)
            e1_nd = e1[:].rearrange("k (n d) -> k n d", n=OW, d=D)
            s2_kn = s2[:].rearrange("b (k n) -> b k n", k=KN, n=OW)

            def squash_v(extra_scale):
                # v2 = squash(vr * extra_scale) over d
                nc.vector.tensor_mul(tmp[:, :KN * D], vr[:], vr[:])
                nc.vector.tensor_reduce(
                    sqv[:].unsqueeze(2),
                    tmp[:, :KN * D].rearrange("b (k d) -> b k d", k=KN, d=D),
                    AX.X, ALU.add)
                es2 = extra_scale * extra_scale
                if es2 != 1.0:
                    nc.vector.tensor_scalar(
                        dnv[:], sqv[:], es2, 1.0, ALU.mult, ALU.add)
                else:
                    nc.vector.tensor_scalar_add(dnv[:], sqv[:], 1.0)
                nc.scalar.activation(rtv[:], sqv[:], ACTF.Sqrt)
                nc.vector.reciprocal(rcv[:], dnv[:])
                # v2 = vr * es2 * sqrt(sq_raw) / (1 + es2*sq_raw)
                nc.vector.scalar_tensor_tensor(
                    scv[:], rtv[:], float(es2), rcv[:], ALU.mult, ALU.mult)
                nc.vector.tensor_mul(
                    v2_kd, vr_kd,
                    scv[:].unsqueeze(2).broadcast_to([B, KN, D]))

            def compute_s2_and_update(first):
                # s2[b,(k,n)] = sum_d T2 * v2(bcast over n)
                nc.vector.tensor_mul(
                    tmp_knd, T2knd,
                    v2_kd.unsqueeze(2).broadcast_to([B, KN, OW, D]))
                nc.vector.tensor_reduce(
                    s2_kn.unsqueeze(3), tmp_knd, AX.X, ALU.add)
                if first:
                    nc.vector.tensor_copy(L[:], s2[:])
                else:
                    nc.vector.tensor_add(L[:], L[:], s2[:])

            # ---- iter 0: uniform probs = 1/16
            nc.vector.tensor_reduce(vr_kd.unsqueeze(3), T2kdn, AX.X, ALU.add)
            squash_v(1.0 / OW)
            compute_s2_and_update(first=True)

            # ---- iters 1, 2
            for it in (1, 2):
                # softmax over n of L[k,n,d]
                nc.vector.tensor_reduce(m1[:].unsqueeze(2), L_dn, AX.X, ALU.max)
                nc.vector.tensor_sub(
                    e1_nd.transpose([0, 2, 1]), L_dn,
                    m1[:].unsqueeze(2).broadcast_to([KN, D, OW]))
                nc.scalar.activation(e1[:], e1[:], ACTF.Exp)
                nc.vector.tensor_reduce(
                    z1[:].unsqueeze(2), e1_nd.transpose([0, 2, 1]), AX.X,
                    ALU.add)
                nc.vector.reciprocal(zr[:], z1[:])
                nc.vector.tensor_mul(
                    P_nd, e1_nd,
                    zr[:].unsqueeze(1).broadcast_to([KN, OW, D]))
                # broadcast P to all 16 b-partitions
                nc.sync.dma_start(Pf[:], P[:])
                for b in range(B):
                    nc.sync.dma_start(Pr[b:b + 1, :], Pf[:])
                # vr[b,(k,d)] = sum_n T2 * Pr
                Pr_kdn = Pr[:].rearrange("b (k n d) -> b k d n",
                                         k=KN, n=OW, d=D)
                nc.vector.tensor_mul(tmp_kdn, T2kdn, Pr_kdn)
                nc.vector.tensor_reduce(
                    vr_kd.unsqueeze(3), tmp_kdn, AX.X, ALU.add)
                squash_v(1.0)
                if it != 2:
                    compute_s2_and_update(first=False)

            nc.sync.dma_start(out_d.ap().rearrange("b k d -> b (k d)"), v2[:])


def _host_prep(x, conv_w, conv_b, prim_w, prim_b, caps_w):
    import ml_dtypes
    xf = np.zeros(B * CI1 * H0 * H0 + XPAD, np.float32)
    xf[:B * CI1 * H0 * H0] = np.ascontiguousarray(x, np.float32).ravel()
    # w1t[j, p=(ci,kh,kwg), co] = conv_w[co, ci, kh, 3*kwg + j]
    w1 = conv_w.reshape(CO1, CI1, KH, NKWG, 3)      # [co, ci, kh, kwg, j]
    w1t = np.ascontiguousarray(
        w1.transpose(4, 1, 2, 3, 0).reshape(NKWG, P81, CO1), np.float32)
    caps5 = caps_w.reshape(B, KN, C2, OH, D)        # c = (cout, oh)
    in_maps = []
    for r in range(NCORES):
        sl = slice(r * COL, (r + 1) * COL)
        w2t = np.ascontiguousarray(
            prim_w[sl].transpose(1, 2, 3, 0).reshape(128, KH * KW * COL),
            np.float32)
        cwt = np.ascontiguousarray(
            caps5[:, :, sl, :, :].transpose(0, 3, 2, 1, 4)
            .reshape(B, OH, COL, KN * D)).astype(ml_dtypes.bfloat16)
        in_maps.append({
            "x": xf,
            "w1t": w1t,
            "b1": np.ascontiguousarray(conv_b, np.float32),
            "w2t": w2t,
            "b2": np.ascontiguousarray(prim_b[sl], np.float32),
            "capst": cwt,
        })
    return in_maps


_compiled = None


def kernel(x, conv_w, conv_b, prim_w, prim_b, caps_w, _trace=False):
    global _compiled
    in_maps = _host_prep(np.asarray(x), np.asarray(conv_w), np.asarray(conv_b),
                         np.asarray(prim_w), np.asarray(prim_b),
                         np.asarray(caps_w))
    if _compiled is None:
        _compiled = _build_program()
    res = bass_utils.run_bass_kernel_spmd(
        _compiled, in_maps, core_ids=list(range(NCORES)), trace=_trace)
    out = res.results[0]["out"].astype(np.float32)
    if _trace:
        return out, res
    return out


# revision 21
# speedup vs baseline: 1.7285x; 1.7285x over previous
"""Trainium2 Bass kernel for nn_CapsuleEncoder (conv stem -> primary caps conv
-> squash -> per-batch routing einsum -> dynamic routing).

Sharding over 8 NeuronCores:
  - conv1 (3->128, 9x9 s1) replicated on every core (tiny).
  - conv2 (128->1024, 9x9 s2) sharded over output channels: 128 couts/core.
    This makes each core own exactly the slice of the routing-einsum
    contraction dim c = (cout, oh) it needs -> tensor-parallel einsum.
  - Partial t = einsum(u, caps_w) AllReduce'd (262 KB) across the 8 cores.
  - Dynamic routing (tiny, 16^4 tensors) replicated on every core.
Matmuls run in bf16 (fp32 PSUM accumulation); pointwise math in fp32.
"""

import sys

sys.path.insert(0, "/opt/trn_rl_repo")

import numpy as np

import concourse.bass as bass
import concourse.bacc as bacc
import concourse.mybir as mybir
from concourse import tile
from concourse import bass_utils

F32 = mybir.dt.float32
BF16 = mybir.dt.bfloat16
AX = mybir.AxisListType
ALU = mybir.AluOpType
ACTF = mybir.ActivationFunctionType

NCORES = 8
B = 16          # batch (== num capsules K by the module's tying)
CI1, CO1 = 3, 128
H0 = 47
KH = KW = 9
H1 = 39         # conv1 output spatial
C2 = 1024       # total primary-caps channels
COL = C2 // NCORES  # 128 couts per core
OH = OW = 16    # conv2 output spatial
KN = 16         # routing out-capsules (einsum 'n')
D = 16          # capsule dim
GB = 2          # batches per conv2/einsum group
NG = B // GB    # 8 groups

# conv1: host-im2col partitions p = (ci, kh, kwg), kwg in 0..2 covering
# kw = 3*kwg + j for pass j in 0..2; rows are flat (oh*47 + ow + j) windows
# into the padded 47-wide image rows, so matmul rhs APs are plain strides.
NKWG = 3
P81 = CI1 * KH * NKWG  # 81
ZROW = H1 * H0         # 1833: flat (oh,47col) row per batch


def _build_program():
    nc = bacc.Bacc("TRN2", target_bir_lowering=False, debug=False,
                   enable_asserts=False, num_devices=NCORES)

    xim_d = nc.dram_tensor("xim", [P81, B * ZROW], BF16, kind="ExternalInput")
    w1_d = nc.dram_tensor("w1t", [NKWG, P81, CO1], BF16, kind="ExternalInput")
    b1_d = nc.dram_tensor("b1", [CO1], F32, kind="ExternalInput")
    w2_d = nc.dram_tensor("w2t", [128, KH * KW * COL], BF16,
                          kind="ExternalInput")
    b2_d = nc.dram_tensor("b2", [COL], F32, kind="ExternalInput")
    cw_d = nc.dram_tensor("capst", [B, OH, COL, KN * D], BF16,
                          kind="ExternalInput")
    out_d = nc.dram_tensor("out", [B, KN, D], F32, kind="ExternalOutput")

    with tile.TileContext(nc) as tc:
        _build_kernel(nc, tc, xim_d, w1_d, b1_d, w2_d, b2_d, cw_d, out_d)
    nc.finalize()
    return nc


def _build_kernel(nc, tc, xim_d, w1_d, b1_d, w2_d, b2_d, cw_d, out_d):
    with (
        tc.tile_pool(name="const", bufs=1) as const_pool,
        tc.tile_pool(name="usq", bufs=3) as usq_pool,
        tc.tile_pool(name="caps", bufs=4) as caps_pool,
        tc.tile_pool(name="sq", bufs=2) as sq_pool,
        tc.tile_pool(name="tsb", bufs=1) as tsb_pool,
        tc.tile_pool(name="ps1", bufs=2, space="PSUM") as ps1_pool,
        tc.tile_pool(name="ps2", bufs=2, space="PSUM") as ps2_pool,
        tc.tile_pool(name="pst", bufs=2, space="PSUM") as pst_pool,
        tc.tile_pool(name="dram", bufs=1, space="DRAM") as dram_pool,
    ):
        # ---------------- constants ----------------
        w1_sb = const_pool.tile([P81, NKWG * CO1], BF16)   # [81, 3*128]
        # DRAM [j, p, co] -> SBUF [p, (j, co)]
        nc.sync.dma_start(
            w1_sb[:].rearrange("p (j co) -> p j co", j=NKWG, co=CO1),
            w1_d.ap().transpose([1, 0, 2]))
        b1_sb = const_pool.tile([CO1, 1], F32)
        nc.sync.dma_start(b1_sb[:], b1_d.ap().unsqueeze(1))
        b2_sb = const_pool.tile([COL, 1], F32)
        nc.sync.dma_start(b2_sb[:], b2_d.ap().unsqueeze(1))

        t_sb = tsb_pool.tile([OW, B * KN * D], F32)  # [w=16, (b, k, d)]

        with tc.tile_pool(name="conv", bufs=1) as conv_pool:
            w2_sb = conv_pool.tile([128, KH * KW * COL], BF16)   # 2.65 MB
            nc.sync.dma_start(w2_sb[:], w2_d.ap())

            # ---------------- conv1 (replicated, all 16 batches) ----------
            # y_sb [ci2=128, (b, ih, iw)] : conv2's contraction layout
            y_sb = conv_pool.tile([CO1, B * H1 * H1], BF16)      # 6.1 MB
            y4 = y_sb[:].rearrange("p (b h w) -> p b h w", b=B, h=H1, w=H1)

            with tc.tile_pool(name="z81", bufs=2) as z_pool:
                for b0 in range(B):
                    z = z_pool.tile([P81, ZROW], BF16, tag="z81")
                    nc.sync.dma_start(
                        z[:], xim_d.ap()[:, b0 * ZROW:(b0 + 1) * ZROW])
                    zv = z[:].rearrange("p (oh c) -> p oh c", oh=H1, c=H0)
                    for ot in range(3):
                        ps = ps1_pool.tile([CO1, 13 * H1], F32, tag="ps1")
                        psv = ps[:].rearrange("p (o w) -> p o w", w=H1)
                        for j in range(NKWG):
                            rhs = zv[:, ot * 13:(ot + 1) * 13, j:j + H1]
                            nc.tensor.matmul(
                                psv[:], w1_sb[:, j * CO1:(j + 1) * CO1], rhs,
                                start=(j == 0), stop=(j == NKWG - 1))
                        off = b0 * (H1 * H1) + ot * 13 * H1
                        nc.scalar.activation(
                            y_sb[:, off:off + 13 * H1], ps[:], ACTF.Relu,
                            bias=b1_sb[:])

            # ---------------- conv2 + squash + einsum pipeline ------------
            usq_tiles = [None] * NG

            def conv2_group(g):
                b0 = g * GB
                ps2 = ps2_pool.tile([COL, GB * OH * OW], F32, tag="ps2")
                first = True
                for kh in range(KH):
                    for kw in range(KW):
                        rhs = y4[:, b0:b0 + GB,
                                 kh:kh + 2 * OH - 1:2, kw:kw + 2 * OW - 1:2]
                        wofs = (kh * KW + kw) * COL
                        nc.tensor.matmul(
                            ps2[:], w2_sb[:, wofs:wofs + COL], rhs,
                            start=first,
                            stop=(kh == KH - 1 and kw == KW - 1))
                        first = False
                # bias-add (psum -> sbuf), then squash over ow
                uw = usq_pool.tile([COL, GB * OH * OW], F32, tag="uw", bufs=2)
                nc.vector.tensor_scalar_add(uw[:], ps2[:], b2_sb[:])
                uw3 = uw[:].rearrange("p (r w) -> p r w", w=OW)  # r=(b,oh)=32
                sqt = sq_pool.tile([COL, GB * OH * OW], F32, tag="sqt")
                nc.vector.tensor_mul(sqt[:], uw[:], uw[:])
                sq = sq_pool.tile([COL, GB * OH], F32, tag="sq")
                nc.vector.tensor_reduce(
                    sq[:].unsqueeze(2),
                    sqt[:].rearrange("p (r w) -> p r w", w=OW), AX.X, ALU.add)
                rt = sq_pool.tile([COL, GB * OH], F32, tag="rtt")
                nc.scalar.activation(rt[:], sq[:], ACTF.Sqrt)
                dn = sq_pool.tile([COL, GB * OH], F32, tag="dn")
                nc.vector.tensor_scalar_add(dn[:], sq[:], 1.0)
                rc = sq_pool.tile([COL, GB * OH], F32, tag="rc")
                nc.vector.reciprocal(rc[:], dn[:])
                sc = sq_pool.tile([COL, GB * OH], F32, tag="sc")
                nc.vector.tensor_mul(sc[:], rt[:], rc[:])
                usq = usq_pool.tile([COL, GB * OH * OW], BF16, tag="usq",
                                    bufs=3)
                nc.vector.tensor_mul(
                    usq[:].rearrange("p (r w) -> p r w", w=OW), uw3,
                    sc[:].unsqueeze(2).broadcast_to([COL, GB * OH, OW]))
                usq_tiles[g] = usq

            def einsum_group(g):
                b0 = g * GB
                usq = usq_tiles[g]
                uv = usq[:].rearrange("p (bb oh w) -> p bb oh w",
                                      bb=GB, oh=OH, w=OW)
                for bb in range(GB):
                    b = b0 + bb
                    ct = caps_pool.tile([COL, OH * KN * D], BF16, tag="caps",
                                        bufs=4)
                    nc.sync.dma_start(
                        ct[:].rearrange("c (oh nd) -> c oh nd", oh=OH),
                        cw_d.ap()[b].transpose([1, 0, 2]))
                    pt = pst_pool.tile([OW, KN * D], F32, tag="pst")
                    for oh in range(OH):
                        nc.tensor.matmul(
                            pt[:], uv[:, bb, oh, :],
                            ct[:, oh * (KN * D):(oh + 1) * (KN * D)],
                            start=(oh == 0), stop=(oh == OH - 1))
                    nc.vector.tensor_copy(
                        t_sb[:, b * (KN * D):(b + 1) * (KN * D)], pt[:])

            for g in range(NG):
                conv2_group(g)
                if g >= 1:
                    einsum_group(g - 1)
            einsum_group(NG - 1)

        # ---------------- AllReduce partial t ----------------
        t_in = dram_pool.tile([B, OW * KN * D], F32)
        t_out = dram_pool.tile([B, OW * KN * D], F32, addr_space="Shared")
        # t_sb [w, (b,k,d)] -> DRAM [b, (w,k,d)]
        nc.sync.dma_start(
            t_in[:].rearrange("b (w kd) -> w b kd", w=OW, kd=KN * D),
            t_sb[:].rearrange("w (b kd) -> w b kd", b=B, kd=KN * D))
        nc.gpsimd.collective_compute(
            "AllReduce", ALU.add,
            replica_groups=[list(range(NCORES))],
            ins=[t_in[:]], outs=[t_out[:]])

        # ---------------- dynamic routing (replicated) ----------------
        with tc.tile_pool(name="rt", bufs=1) as rt_pool:
            # T_full[b][n][k][d] with n = spatial w. T2 [b(part), (k,n,d)],
            # free strides k:256, n:16, d:1.
            T2 = rt_pool.tile([B, KN * OW * D], F32)
            Traw = rt_pool.tile([B, OW * KN * D], F32)
            nc.sync.dma_start(Traw[:], t_out[:])
            nc.vector.tensor_copy(
                T2[:].rearrange("b (k n d) -> b n k d", k=KN, n=OW, d=D),
                Traw[:].rearrange("b (n k d) -> b n k d", n=OW, k=KN, d=D))
            T2knd = T2[:].rearrange("b (k n d) -> b k n d", k=KN, n=OW, d=D)
            T2kdn = T2knd.transpose([0, 1, 3, 2])

            L = rt_pool.tile([KN, OW * D], F32)    # logits(k,n,d); (n,d)=(16,1)
            P = rt_pool.tile([KN, OW * D], F32)    # probs, same layout
            Pf = rt_pool.tile([1, KN * OW * D], F32)
            Pr = rt_pool.tile([B, KN * OW * D], F32)  # probs bcast over b
            tmp = rt_pool.tile([B, KN * OW * D], F32)
            vr = rt_pool.tile([B, KN * D], F32)    # raw out, (k,d) = (16,1)
            v2 = rt_pool.tile([B, KN * D], F32)
            s2 = rt_pool.tile([B, KN * OW], F32)   # logit delta, (k,n)=(16,1)
            sqv = rt_pool.tile([B, KN], F32)
            rtv = rt_pool.tile([B, KN], F32)
            dnv = rt_pool.tile([B, KN], F32)
            rcv = rt_pool.tile([B, KN], F32)
            scv = rt_pool.tile([B, KN], F32)
            m1 = rt_pool.tile([KN, D], F32)
            e1 = rt_pool.tile([KN, OW * D], F32)
            z1 = rt_pool.tile([KN, D], F32)
            zr = rt_pool.tile([KN, D], F32)

            vr_kd = vr[:].rearrange("b (k d) -> b k d", k=KN, d=D)
            v2_kd = v2[:].rearrange("b (k d) -> b k d", k=KN, d=D)
            tmp_kdn = tmp[:].rearrange("b (k d n) -> b k d n", k=KN, d=D, n=OW)
            tmp_knd = tmp[:].rearrange("b (k n d) -> b k n d", k=KN, n=OW, d=D)
            L_nd = L[:].rearrange("k (n d) -> k n d", n=OW, d=D)
            L_dn = L_nd.transpose([0, 2, 1])
            P_nd = P[:].rearrange("k (n d) -> k n d", n=OW, d=D)
            e1_nd = e1[:].rearrange("k (n d) -> k n d", n=OW, d=D)
            s2_kn = s2[:].rearrange("b (k n) -> b k n", k=KN, n=OW)

            def squash_v(extra_scale):
                # v2 = squash(vr * extra_scale) over d
                nc.vector.tensor_mul(tmp[:, :KN * D], vr[:], vr[:])
                nc.vector.tensor_reduce(
                    sqv[:].unsqueeze(2),
                    tmp[:, :KN * D].rearrange("b (k d) -> b k d", k=KN, d=D),
                    AX.X, ALU.add)
                es2 = extra_scale * extra_scale
                if es2 != 1.0:
                    nc.vector.tensor_scalar(
                        dnv[:], sqv[:], es2, 1.0, ALU.mult, ALU.add)
                else:
                    nc.vector.tensor_scalar_add(dnv[:], sqv[:], 1.0)
                nc.scalar.activation(rtv[:], sqv[:], ACTF.Sqrt)
                nc.vector.reciprocal(rcv[:], dnv[:])
                # v2 = vr * es2 * sqrt(sq_raw) / (1 + es2*sq_raw)
                nc.vector.scalar_tensor_tensor(
                    scv[:], rtv[:], float(es2), rcv[:], ALU.mult, ALU.mult)
                nc.vector.tensor_mul(
                    v2_kd, vr_kd,
                    scv[:].unsqueeze(2).broadcast_to([B, KN, D]))

            def compute_s2_and_update(first):
                # s2[b,(k,n)] = sum_d T2 * v2(bcast over n)
                nc.vector.tensor_mul(
                    tmp_knd, T2knd,
                    v2_kd.unsqueeze(2).broadcast_to([B, KN, OW, D]))
                nc.vector.tensor_reduce(
                    s2_kn.unsqueeze(3), tmp_knd, AX.X, ALU.add)
                if first:
                    nc.vector.tensor_copy(L[:], s2[:])
                else:
                    nc.vector.tensor_add(L[:], L[:], s2[:])

            # ---- iter 0: uniform probs = 1/16
            nc.vector.tensor_reduce(vr_kd.unsqueeze(3), T2kdn, AX.X, ALU.add)
            squash_v(1.0 / OW)
            compute_s2_and_update(first=True)

            # ---- iters 1, 2
            for it in (1, 2):
                # softmax over n of L[k,n,d]
                nc.vector.tensor_reduce(m1[:].unsqueeze(2), L_dn, AX.X, ALU.max)
                nc.vector.tensor_sub(
                    e1_nd.transpose([0, 2, 1]), L_dn,
                    m1[:].unsqueeze(2).broadcast_to([KN, D, OW]))
                nc.scalar.activation(e1[:], e1[:], ACTF.Exp)
                nc.vector.tensor_reduce(
                    z1[:].unsqueeze(2), e1_nd.transpose([0, 2, 1]), AX.X,
                    ALU.add)
                nc.vector.reciprocal(zr[:], z1[:])
                nc.vector.tensor_mul(
                    P_nd, e1_nd,
                    zr[:].unsqueeze(1).broadcast_to([KN, OW, D]))
                # broadcast P to all 16 b-partitions
                nc.sync.dma_start(Pf[:], P[:])
                for b in range(B):
                    nc.sync.dma_start(Pr[b:b + 1, :], Pf[:])
                # vr[b,(k,d)] = sum_n T2 * Pr
                Pr_kdn = Pr[:].rearrange("b (k n d) -> b k d n",
                                         k=KN, n=OW, d=D)
                nc.vector.tensor_mul(tmp_kdn, T2kdn, Pr_kdn)
                nc.vector.tensor_reduce(
                    vr_kd.unsqueeze(3), tmp_kdn, AX.X, ALU.add)
                squash_v(1.0)
                if it != 2:
                    compute_s2_and_update(first=False)

            nc.sync.dma_start(out_d.ap().rearrange("b k d -> b (k d)"), v2[:])


def _host_prep(x, conv_w, conv_b, prim_w, prim_b, caps_w):
    import ml_dtypes
    bf16 = ml_dtypes.bfloat16
    x = np.ascontiguousarray(x, np.float32)
    # host im2col for conv1: xim[p=(ci,kh,kwg)][b, oh*47+col] =
    #   x[b, ci, kh+oh, col+3*kwg] laid out as flat shifted windows of the
    #   row-major image so device-side matmul rhs APs are plain strides.
    xpad = np.zeros((B, CI1, H0 * H0 + 8), np.float32)
    xpad[:, :, :H0 * H0] = x.reshape(B, CI1, H0 * H0)
    xim = np.empty((P81, B, ZROW), np.float32)
    for ci in range(CI1):
        for kh in range(KH):
            for kwg in range(NKWG):
                p = ci * (KH * NKWG) + kh * NKWG + kwg
                s = kh * H0 + 3 * kwg
                xim[p] = xpad[:, ci, s:s + ZROW]
    xim_b = np.ascontiguousarray(xim.reshape(P81, B * ZROW)).astype(bf16)
    # w1t[j, p=(ci,kh,kwg), co] = conv_w[co, ci, kh, 3*kwg + j]
    w1 = conv_w.reshape(CO1, CI1, KH, NKWG, 3)      # [co, ci, kh, kwg, j]
    w1t = np.ascontiguousarray(
        w1.transpose(4, 1, 2, 3, 0).reshape(NKWG, P81, CO1)).astype(bf16)
    caps5 = caps_w.reshape(B, KN, C2, OH, D)        # c = (cout, oh)
    in_maps = []
    for r in range(NCORES):
        sl = slice(r * COL, (r + 1) * COL)
        w2t = np.ascontiguousarray(
            prim_w[sl].transpose(1, 2, 3, 0).reshape(128, KH * KW * COL)
        ).astype(bf16)
        cwt = np.ascontiguousarray(
            caps5[:, :, sl, :, :].transpose(0, 3, 2, 1, 4)
            .reshape(B, OH, COL, KN * D)).astype(bf16)
        in_maps.append({
            "xim": xim_b,
            "w1t": w1t,
            "b1": np.ascontiguousarray(conv_b, np.float32),
            "w2t": w2t,
            "b2": np.ascontiguousarray(prim_b[sl], np.float32),
            "capst": cwt,
        })
    return in_maps


_compiled = None


def kernel(x, conv_w, conv_b, prim_w, prim_b, caps_w, _trace=False):
    global _compiled
    in_maps = _host_prep(np.asarray(x), np.asarray(conv_w), np.asarray(conv_b),
                         np.asarray(prim_w), np.asarray(prim_b),
                         np.asarray(caps_w))
    if _compiled is None:
        _compiled = _build_program()
    res = bass_utils.run_bass_kernel_spmd(
        _compiled, in_maps, core_ids=list(range(NCORES)), trace=_trace)
    out = res.results[0]["out"].astype(np.float32)
    if _trace:
        return out, res
    return out


# revision 23
# speedup vs baseline: 1.8656x; 1.0793x over previous
"""Trainium2 Bass kernel for nn_CapsuleEncoder (conv stem -> primary caps conv
-> squash -> per-batch routing einsum -> dynamic routing).

Sharding over 8 NeuronCores:
  - conv1 (3->128, 9x9 s1) replicated on every core (tiny).
  - conv2 (128->1024, 9x9 s2) sharded over output channels: 128 couts/core.
    This makes each core own exactly the slice of the routing-einsum
    contraction dim c = (cout, oh) it needs -> tensor-parallel einsum.
  - Partial t = einsum(u, caps_w) AllReduce'd (262 KB) across the 8 cores.
  - Dynamic routing (tiny, 16^4 tensors) replicated on every core.
Matmuls run in bf16 (fp32 PSUM accumulation); pointwise math in fp32.
"""

import sys

sys.path.insert(0, "/opt/trn_rl_repo")

import numpy as np

import concourse.bass as bass
import concourse.bacc as bacc
import concourse.mybir as mybir
from concourse import tile
from concourse import bass_utils

F32 = mybir.dt.float32
BF16 = mybir.dt.bfloat16
AX = mybir.AxisListType
ALU = mybir.AluOpType
ACTF = mybir.ActivationFunctionType

NCORES = 8
B = 16          # batch (== num capsules K by the module's tying)
CI1, CO1 = 3, 128
H0 = 47
KH = KW = 9
H1 = 39         # conv1 output spatial
C2 = 1024       # total primary-caps channels
COL = C2 // NCORES  # 128 couts per core
OH = OW = 16    # conv2 output spatial
KN = 16         # routing out-capsules (einsum 'n')
D = 16          # capsule dim
GB = 2          # batches per conv2/einsum group
NG = B // GB    # 8 groups

# conv1: host-im2col partitions p = (ci, kh, kwg), kwg in 0..2 covering
# kw = 3*kwg + j for pass j in 0..2; rows are flat (oh*47 + ow + j) windows
# into the padded 47-wide image rows, so matmul rhs APs are plain strides.
NKWG = 3
P81 = CI1 * KH * NKWG  # 81
ZROW = H1 * H0         # 1833: flat (oh,47col) row per batch


def _build_program():
    nc = bacc.Bacc("TRN2", target_bir_lowering=False, debug=False,
                   enable_asserts=False, num_devices=NCORES)

    xim_d = nc.dram_tensor("xim", [P81, B * ZROW], BF16, kind="ExternalInput")
    w1_d = nc.dram_tensor("w1t", [NKWG, P81, CO1], BF16, kind="ExternalInput")
    b1_d = nc.dram_tensor("b1", [CO1], F32, kind="ExternalInput")
    w2_d = nc.dram_tensor("w2t", [128, KH * KW * COL], BF16,
                          kind="ExternalInput")
    b2_d = nc.dram_tensor("b2", [COL], F32, kind="ExternalInput")
    cw_d = nc.dram_tensor("capst", [B, OH, COL, KN * D], BF16,
                          kind="ExternalInput")
    out_d = nc.dram_tensor("out", [B, KN, D], F32, kind="ExternalOutput")

    with tile.TileContext(nc) as tc:
        _build_kernel(nc, tc, xim_d, w1_d, b1_d, w2_d, b2_d, cw_d, out_d)
    nc.finalize()
    return nc


def _build_kernel(nc, tc, xim_d, w1_d, b1_d, w2_d, b2_d, cw_d, out_d):
    with (
        tc.tile_pool(name="const", bufs=1) as const_pool,
        tc.tile_pool(name="usq", bufs=3) as usq_pool,
        tc.tile_pool(name="caps", bufs=4) as caps_pool,
        tc.tile_pool(name="sq", bufs=2) as sq_pool,
        tc.tile_pool(name="tsb", bufs=1) as tsb_pool,
        tc.tile_pool(name="ps1", bufs=2, space="PSUM") as ps1_pool,
        tc.tile_pool(name="ps2", bufs=2, space="PSUM") as ps2_pool,
        tc.tile_pool(name="pst", bufs=2, space="PSUM") as pst_pool,
        tc.tile_pool(name="dram", bufs=1, space="DRAM") as dram_pool,
    ):
        # ---------------- constants ----------------
        w1_sb = const_pool.tile([P81, NKWG * CO1], BF16)   # [81, 3*128]
        # DRAM [j, p, co] -> SBUF [p, (j, co)]
        nc.sync.dma_start(
            w1_sb[:].rearrange("p (j co) -> p j co", j=NKWG, co=CO1),
            w1_d.ap().transpose([1, 0, 2]))
        b1_sb = const_pool.tile([CO1, 1], F32)
        nc.sync.dma_start(b1_sb[:], b1_d.ap().unsqueeze(1))
        b2_sb = const_pool.tile([COL, 1], F32)
        nc.sync.dma_start(b2_sb[:], b2_d.ap().unsqueeze(1))

        t_sb = tsb_pool.tile([OW, B * KN * D], F32)  # [w=16, (b, k, d)]

        with tc.tile_pool(name="conv", bufs=1) as conv_pool:
            w2_sb = conv_pool.tile([128, KH * KW * COL], BF16)   # 2.65 MB
            nc.sync.dma_start(w2_sb[:], w2_d.ap())

            # ---------------- conv1 (replicated, all 16 batches) ----------
            # y_sb [ci2=128, (b, ih, iw)] : conv2's contraction layout
            y_sb = conv_pool.tile([CO1, B * H1 * H1], BF16)      # 6.1 MB
            y4 = y_sb[:].rearrange("p (b h w) -> p b h w", b=B, h=H1, w=H1)

            with tc.tile_pool(name="z81", bufs=2) as z_pool:
                for b0 in range(B):
                    z = z_pool.tile([P81, ZROW], BF16, tag="z81")
                    nc.sync.dma_start(
                        z[:], xim_d.ap()[:, b0 * ZROW:(b0 + 1) * ZROW])
                    zv = z[:].rearrange("p (oh c) -> p oh c", oh=H1, c=H0)
                    for ot in range(3):
                        ps = ps1_pool.tile([CO1, 13 * H1], F32, tag="ps1")
                        psv = ps[:].rearrange("p (o w) -> p o w", w=H1)
                        for j in range(NKWG):
                            rhs = zv[:, ot * 13:(ot + 1) * 13, j:j + H1]
                            nc.tensor.matmul(
                                psv[:], w1_sb[:, j * CO1:(j + 1) * CO1], rhs,
                                start=(j == 0), stop=(j == NKWG - 1))
                        off = b0 * (H1 * H1) + ot * 13 * H1
                        nc.scalar.activation(
                            y_sb[:, off:off + 13 * H1], ps[:], ACTF.Relu,
                            bias=b1_sb[:])

            # ---------------- conv2 + squash + einsum pipeline ------------
            usq_tiles = [None] * NG

            def conv2_group(g):
                b0 = g * GB
                ps2 = ps2_pool.tile([COL, GB * OH * OW], F32, tag="ps2")
                first = True
                for kh in range(KH):
                    for kw in range(KW):
                        rhs = y4[:, b0:b0 + GB,
                                 kh:kh + 2 * OH - 1:2, kw:kw + 2 * OW - 1:2]
                        wofs = (kh * KW + kw) * COL
                        nc.tensor.matmul(
                            ps2[:], w2_sb[:, wofs:wofs + COL], rhs,
                            start=first,
                            stop=(kh == KH - 1 and kw == KW - 1))
                        first = False
                # bias-add (psum -> sbuf), then squash over ow
                uw = usq_pool.tile([COL, GB * OH * OW], F32, tag="uw", bufs=2)
                nc.vector.tensor_scalar_add(uw[:], ps2[:], b2_sb[:])
                uw3 = uw[:].rearrange("p (r w) -> p r w", w=OW)  # r=(b,oh)=32
                sqt = sq_pool.tile([COL, GB * OH * OW], F32, tag="sqt")
                nc.vector.tensor_mul(sqt[:], uw[:], uw[:])
                sq = sq_pool.tile([COL, GB * OH], F32, tag="sq")
                nc.vector.tensor_reduce(
                    sq[:].unsqueeze(2),
                    sqt[:].rearrange("p (r w) -> p r w", w=OW), AX.X, ALU.add)
                rt = sq_pool.tile([COL, GB * OH], F32, tag="rtt")
                nc.scalar.activation(rt[:], sq[:], ACTF.Sqrt)
                dn = sq_pool.tile([COL, GB * OH], F32, tag="dn")
                nc.vector.tensor_scalar_add(dn[:], sq[:], 1.0)
                rc = sq_pool.tile([COL, GB * OH], F32, tag="rc")
                nc.vector.reciprocal(rc[:], dn[:])
                sc = sq_pool.tile([COL, GB * OH], F32, tag="sc")
                nc.vector.tensor_mul(sc[:], rt[:], rc[:])
                usq = usq_pool.tile([COL, GB * OH * OW], BF16, tag="usq",
                                    bufs=3)
                nc.vector.tensor_mul(
                    usq[:].rearrange("p (r w) -> p r w", w=OW), uw3,
                    sc[:].unsqueeze(2).broadcast_to([COL, GB * OH, OW]))
                usq_tiles[g] = usq

            def einsum_group(g):
                b0 = g * GB
                usq = usq_tiles[g]
                uv = usq[:].rearrange("p (bb oh w) -> p bb oh w",
                                      bb=GB, oh=OH, w=OW)
                for bb in range(GB):
                    b = b0 + bb
                    ct = caps_pool.tile([COL, OH * KN * D], BF16, tag="caps",
                                        bufs=4)
                    nc.sync.dma_start(
                        ct[:].rearrange("c (oh nd) -> c oh nd", oh=OH),
                        cw_d.ap()[b].transpose([1, 0, 2]))
                    pt = pst_pool.tile([OW, KN * D], F32, tag="pst")
                    for oh in range(OH):
                        nc.tensor.matmul(
                            pt[:], uv[:, bb, oh, :],
                            ct[:, oh * (KN * D):(oh + 1) * (KN * D)],
                            start=(oh == 0), stop=(oh == OH - 1))
                    nc.vector.tensor_copy(
                        t_sb[:, b * (KN * D):(b + 1) * (KN * D)], pt[:])

            HB = B // 2
            t_in = dram_pool.tile([B, OW * KN * D], F32)
            t_out0 = dram_pool.tile([HB, OW * KN * D], F32,
                                    addr_space="Shared")
            t_out1 = dram_pool.tile([HB, OW * KN * D], F32,
                                    addr_space="Shared")
            t_outs = [t_out0, t_out1]
            t_sb_v = t_sb[:].rearrange("w (b kd) -> w b kd", b=B, kd=KN * D)
            t_in_v = t_in[:].rearrange("b (w kd) -> w b kd", w=OW, kd=KN * D)

            def allreduce_half(h):
                bs = slice(h * HB, (h + 1) * HB)
                nc.sync.dma_start(t_in_v[:, bs], t_sb_v[:, bs])
                nc.gpsimd.collective_compute(
                    "AllReduce", ALU.add,
                    replica_groups=[list(range(NCORES))],
                    ins=[t_in[bs, :]], outs=[t_outs[h][:]])

            for g in range(NG):
                conv2_group(g)
                if g >= 1:
                    einsum_group(g - 1)
                if g == NG - 1:
                    allreduce_half(0)
            einsum_group(NG - 1)
            allreduce_half(1)

        # ---------------- dynamic routing (replicated) ----------------
        with tc.tile_pool(name="rt", bufs=1) as rt_pool:
            # T_full[b][n][k][d] with n = spatial w. T2 [b(part), (k,n,d)],
            # free strides k:256, n:16, d:1.
            T2 = rt_pool.tile([B, KN * OW * D], F32)
            Traw = rt_pool.tile([B, OW * KN * D], F32)
            nc.sync.dma_start(Traw[0:HB, :], t_out0[:])
            nc.sync.dma_start(Traw[HB:B, :], t_out1[:])
            nc.vector.tensor_copy(
                T2[:].rearrange("b (k n d) -> b n k d", k=KN, n=OW, d=D),
                Traw[:].rearrange("b (n k d) -> b n k d", n=OW, k=KN, d=D))
            T2knd = T2[:].rearrange("b (k n d) -> b k n d", k=KN, n=OW, d=D)
            T2kdn = T2knd.transpose([0, 1, 3, 2])

            L = rt_pool.tile([KN, OW * D], F32)    # logits(k,n,d); (n,d)=(16,1)
            P = rt_pool.tile([KN, OW * D], F32)    # probs, same layout
            Pf16 = rt_pool.tile([B, KN * OW * D], F32)
            Pr = rt_pool.tile([B, KN * OW * D], F32)  # probs bcast over b
            tmp = rt_pool.tile([B, KN * OW * D], F32)
            vr = rt_pool.tile([B, KN * D], F32)    # raw out, (k,d) = (16,1)
            v2 = rt_pool.tile([B, KN * D], F32)
            s2 = rt_pool.tile([B, KN * OW], F32)   # logit delta, (k,n)=(16,1)
            sqv = rt_pool.tile([B, KN], F32)
            rtv = rt_pool.tile([B, KN], F32)
            dnv = rt_pool.tile([B, KN], F32)
            rcv = rt_pool.tile([B, KN], F32)
            scv = rt_pool.tile([B, KN], F32)
            e1 = rt_pool.tile([KN, OW * D], F32)
            z1 = rt_pool.tile([KN, D], F32)
            zr = rt_pool.tile([KN, D], F32)

            vr_kd = vr[:].rearrange("b (k d) -> b k d", k=KN, d=D)
            v2_kd = v2[:].rearrange("b (k d) -> b k d", k=KN, d=D)
            tmp_kdn = tmp[:].rearrange("b (k d n) -> b k d n", k=KN, d=D, n=OW)
            tmp_knd = tmp[:].rearrange("b (k n d) -> b k n d", k=KN, n=OW, d=D)
            L_nd = L[:].rearrange("k (n d) -> k n d", n=OW, d=D)
            P_nd = P[:].rearrange("k (n d) -> k n d", n=OW, d=D)
            e1_nd = e1[:].rearrange("k (n d) -> k n d", n=OW, d=D)
            s2_kn = s2[:].rearrange("b (k n) -> b k n", k=KN, n=OW)

            def squash_v(extra_scale):
                # v2 = squash(vr * extra_scale) over d
                nc.vector.tensor_mul(tmp[:, :KN * D], vr[:], vr[:])
                nc.vector.tensor_reduce(
                    sqv[:].unsqueeze(2),
                    tmp[:, :KN * D].rearrange("b (k d) -> b k d", k=KN, d=D),
                    AX.X, ALU.add)
                es2 = extra_scale * extra_scale
                if es2 != 1.0:
                    nc.vector.tensor_scalar(
                        dnv[:], sqv[:], es2, 1.0, ALU.mult, ALU.add)
                else:
                    nc.vector.tensor_scalar_add(dnv[:], sqv[:], 1.0)
                nc.scalar.activation(rtv[:], sqv[:], ACTF.Sqrt)
                nc.vector.reciprocal(rcv[:], dnv[:])
                # v2 = vr * es2 * sqrt(sq_raw) / (1 + es2*sq_raw)
                nc.vector.scalar_tensor_tensor(
                    scv[:], rtv[:], float(es2), rcv[:], ALU.mult, ALU.mult)
                nc.vector.tensor_mul(
                    v2_kd, vr_kd,
                    scv[:].unsqueeze(2).broadcast_to([B, KN, D]))

            def compute_s2_and_update(first):
                # s2[b,(k,n)] = sum_d T2 * v2(bcast over n)
                nc.vector.tensor_mul(
                    tmp_knd, T2knd,
                    v2_kd.unsqueeze(2).broadcast_to([B, KN, OW, D]))
                nc.vector.tensor_reduce(
                    s2_kn.unsqueeze(3), tmp_knd, AX.X, ALU.add)
                if first:
                    nc.vector.tensor_copy(L[:], s2[:])
                else:
                    nc.vector.tensor_add(L[:], L[:], s2[:])

            # ---- iter 0: uniform probs = 1/16
            nc.vector.tensor_reduce(vr_kd.unsqueeze(3), T2kdn, AX.X, ALU.add)
            squash_v(1.0 / OW)
            compute_s2_and_update(first=True)

            # ---- iters 1, 2
            for it in (1, 2):
                # softmax over n of L[k,n,d] (logits are O(1), skip max-sub)
                nc.scalar.activation(e1[:], L[:], ACTF.Exp)
                nc.vector.tensor_reduce(
                    z1[:].unsqueeze(2), e1_nd.transpose([0, 2, 1]), AX.X,
                    ALU.add)
                nc.vector.reciprocal(zr[:], z1[:])
                nc.vector.tensor_mul(
                    P_nd, e1_nd,
                    zr[:].unsqueeze(1).broadcast_to([KN, OW, D]))
                # broadcast P to all 16 b-partitions via DVE shuffle
                nc.sync.dma_start(Pf16[0:1, :], P[:])
                nc.vector.stream_shuffle(Pr[:], Pf16[:16, :], [0] * 32)
                # vr[b,(k,d)] = sum_n T2 * Pr
                Pr_kdn = Pr[:].rearrange("b (k n d) -> b k d n",
                                         k=KN, n=OW, d=D)
                nc.vector.tensor_mul(tmp_kdn, T2kdn, Pr_kdn)
                nc.vector.tensor_reduce(
                    vr_kd.unsqueeze(3), tmp_kdn, AX.X, ALU.add)
                squash_v(1.0)
                if it != 2:
                    compute_s2_and_update(first=False)

            nc.sync.dma_start(out_d.ap().rearrange("b k d -> b (k d)"), v2[:])


def _host_prep(x, conv_w, conv_b, prim_w, prim_b, caps_w):
    import ml_dtypes
    bf16 = ml_dtypes.bfloat16
    x = np.ascontiguousarray(x, np.float32)
    # host im2col for conv1: xim[p=(ci,kh,kwg)][b, oh*47+col] =
    #   x[b, ci, kh+oh, col+3*kwg] laid out as flat shifted windows of the
    #   row-major image so device-side matmul rhs APs are plain strides.
    xpad = np.zeros((B, CI1, H0 * H0 + 8), np.float32)
    xpad[:, :, :H0 * H0] = x.reshape(B, CI1, H0 * H0)
    xim = np.empty((P81, B, ZROW), np.float32)
    for ci in range(CI1):
        for kh in range(KH):
            for kwg in range(NKWG):
                p = ci * (KH * NKWG) + kh * NKWG + kwg
                s = kh * H0 + 3 * kwg
                xim[p] = xpad[:, ci, s:s + ZROW]
    xim_b = np.ascontiguousarray(xim.reshape(P81, B * ZROW)).astype(bf16)
    # w1t[j, p=(ci,kh,kwg), co] = conv_w[co, ci, kh, 3*kwg + j]
    w1 = conv_w.reshape(CO1, CI1, KH, NKWG, 3)      # [co, ci, kh, kwg, j]
    w1t = np.ascontiguousarray(
        w1.transpose(4, 1, 2, 3, 0).reshape(NKWG, P81, CO1)).astype(bf16)
    caps5 = caps_w.reshape(B, KN, C2, OH, D)        # c = (cout, oh)
    in_maps = []
    for r in range(NCORES):
        sl = slice(r * COL, (r + 1) * COL)
        w2t = np.ascontiguousarray(
            prim_w[sl].transpose(1, 2, 3, 0).reshape(128, KH * KW * COL)
        ).astype(bf16)
        cwt = np.ascontiguousarray(
            caps5[:, :, sl, :, :].transpose(0, 3, 2, 1, 4)
            .reshape(B, OH, COL, KN * D)).astype(bf16)
        in_maps.append({
            "xim": xim_b,
            "w1t": w1t,
            "b1": np.ascontiguousarray(conv_b, np.float32),
            "w2t": w2t,
            "b2": np.ascontiguousarray(prim_b[sl], np.float32),
            "capst": cwt,
        })
    return in_maps


_compiled = None


def kernel(x, conv_w, conv_b, prim_w, prim_b, caps_w, _trace=False):
    global _compiled
    in_maps = _host_prep(np.asarray(x), np.asarray(conv_w), np.asarray(conv_b),
                         np.asarray(prim_w), np.asarray(prim_b),
                         np.asarray(caps_w))
    if _compiled is None:
        _compiled = _build_program()
    res = bass_utils.run_bass_kernel_spmd(
        _compiled, in_maps, core_ids=list(range(NCORES)), trace=_trace)
    out = res.results[0]["out"].astype(np.float32)
    if _trace:
        return out, res
    return out
